# revision 1
# baseline (speedup 1.0000x reference)
"""Trainium2 Bass kernel for nn_AdapterAttnForMamba (depthwise 3x3 conv over a
pad-token-augmented 66x66 image + spatial-transpose permutation + residual).

Math (per batch b, channel c), derived from the reference:
  out(i,j) = x(i,j) + y(j,i) + bias_c
  y(r,s)   = sum_{a,b in 0..2} w[c,a,b] * V[r+a-1, s+b-1]
  V        = 65x65 "virtual" image: V[i<64, j<64] = x(i,j); V[i<64, 64] = tok0;
             V[64, j] = tok_{j%2}; zero outside (SAME conv padding).

Strategy: pure data parallel over batch (2 batches / core, 8 cores).
On-chip layout "A" (channels on partitions) obtained via TensorE transposes;
conv taps are diagonal matmuls accumulating in PSUM (tap shifts are free-dim
AP offsets into a padded SBUF image P). The seq_idx spatial transpose is folded
into the tap access pattern (we read V[j+di, i+dj] directly), so all DMAs stay
in natural, contiguous layout.

Pipeline per (batch bi, channel-block ct of 128):
  x stripes [128pix,1024ch] --(SWDGE cast f32->bf16)--> SBUF
  TensorE transpose -> PSUM [128ch,128pix] -> ACT copy -> P (padded, row stride 80)
  9 diag-matmuls (bf16) accumulate conv into PSUM chunks [128,512]
  ACT evac + conv_b bias -> z[ct] (bf16, l-order = i-major pixels)
  TensorE transpose back -> PSUM [128pix,1024ch]; DVE adds residual x stripe
  SWDGE cast bf16->f32 DMA to DRAM out.
"""

import os
import sys

import numpy as np

for _p in ("/opt/trn_rl_repo", "/root/.axon_site/_ro/trn_rl_repo"):
    if os.path.isdir(_p) and _p not in sys.path:
        sys.path.append(_p)

B, H, W, C = 16, 64, 64, 1024
L = H * W  # 4096
NCORES = 8
BPC = B // NCORES  # batches per core
NCT = C // 128  # channel blocks
RS = 80  # P row stride (elements); >= 67 and 16-aligned (fp8-ready)
PROWS = 66  # P rows: image rows -1..64 (+1 ring)
PSZ = PROWS * RS
NSTRIPE = L // 128  # 32 pixel stripes per batch
TAPS = [(di, dj) for di in (-1, 0, 1) for dj in (-1, 0, 1)]

_CACHE = {}


def _expected_seq_idx():
    return np.arange(L).reshape(H, W).T.reshape(-1)


def _build_nc():
    import concourse.mybir as mybir
    from concourse import bacc
    from concourse.masks import make_identity
    from concourse.tile import TileContext

    f32 = mybir.dt.float32
    f32r = mybir.dt.float32r
    bf16 = mybir.dt.bfloat16
    Copy = mybir.ActivationFunctionType.Copy
    Ident = mybir.ActivationFunctionType.Identity

    nc = bacc.Bacc(None, target_bir_lowering=False)
    x_ext = nc.declare_dram_parameter("x", [BPC, L, C], f32, isOutput=False)
    tok_ext = nc.declare_dram_parameter("pad_token", [1, C, 2], f32, isOutput=False)
    w_ext = nc.declare_dram_parameter("conv_w", [C, 1, 3, 3], f32, isOutput=False)
    b_ext = nc.declare_dram_parameter("conv_b", [C], f32, isOutput=False)
    out_ext = nc.declare_dram_parameter("out", [BPC, L, C], f32, isOutput=True)

    with TileContext(nc) as tc:
        with (
            tc.tile_pool(name="const", bufs=1) as constp,
            tc.tile_pool(name="xpool", bufs=32) as xpool,
            tc.tile_pool(name="zpool", bufs=8) as zpool,
            tc.tile_pool(name="qpool", bufs=2) as qpool,
            tc.tile_pool(name="tpool", bufs=3) as tpool,
            tc.tile_pool(name="opool", bufs=2) as opool,
            tc.tile_pool(name="dpool", bufs=12) as dpool,
            tc.tile_pool(name="ps_t", bufs=2, space="PSUM") as ps_t,
            tc.tile_pool(name="ps_z", bufs=2, space="PSUM") as ps_z,
            tc.tile_pool(name="ps_o", bufs=2, space="PSUM") as ps_o,
        ):
            # ---- constants ----
            ident = constp.tile([128, 128], bf16, tag="ident")
            make_identity(nc, ident)
            zeros = constp.tile([128, 128], bf16, tag="zeros")
            nc.vector.memset(zeros[:], 0.0)

            wt = constp.tile([128, 9 * NCT], f32, tag="wt")
            cb = constp.tile([128, NCT], f32, tag="cb")
            tokt = constp.tile([128, 2 * NCT], f32, tag="tokt")
            nc.sync.dma_start(
                out=wt.rearrange("p (ct t) -> p ct t", t=9),
                in_=w_ext.rearrange("(ct p) a k l -> p ct (a k l)", ct=NCT),
            )
            nc.sync.dma_start(
                out=cb[:],
                in_=b_ext.rearrange("(ct p) -> p ct", ct=NCT),
            )
            nc.sync.dma_start(
                out=tokt.rearrange("p (ct two) -> p ct two", two=2),
                in_=tok_ext.rearrange("a (ct p) two -> p ct (a two)", ct=NCT),
            )

            # ---- main loops ----
            # Layouts: x tiles and z are in natural l-order (i-major).
            # Q holds the padded image TRANSPOSED: Q[p_ch, (i+1)*RS + (j+1)]
            # = V[j, i]  (f32r so tap matmuls run 1 cyc/row with unit-stride
            # inner runs; 4-byte dtype avoids the bf16 odd-offset AP bug).
            for bi in range(BPC):
                xs = []
                for s in range(NSTRIPE):
                    xt = xpool.tile([128, C], bf16, tag="x")
                    # SWDGE dma casts f32 -> bf16; plain natural stripe
                    nc.gpsimd.dma_start(
                        out=xt[:], in_=x_ext[bi, s * 128 : (s + 1) * 128, :]
                    )
                    xs.append(xt)

                z_tiles = []
                for ct in range(NCT):
                    # -------- phase A: build transposed padded image Q ------
                    Q = qpool.tile([128, PSZ], f32r, tag="Q")
                    Qv = Q.rearrange("p (r c) -> p r c", c=RS)
                    # ring zeros via ACT (memset can't produce f32r)
                    nc.scalar.activation(
                        out=Qv[:, 0:1, 0:67], in_=zeros[:, 0:67], func=Copy, scale=1.0
                    )  # Q row 0 = V[:, -1]
                    nc.scalar.activation(
                        out=Qv[:, 1:66, 0:1], in_=zeros[:, 0:65], func=Copy, scale=1.0
                    )  # Q col 0 = V[-1, :]
                    diag = []
                    for t in range(9):
                        d = dpool.tile([128, 128], f32r, tag="diag")
                        nc.vector.tensor_scalar_mul(
                            out=d[:],
                            in0=ident[:],
                            scalar1=wt[:, ct * 9 + t : ct * 9 + t + 1],
                        )
                        diag.append(d)
                    for g in range(8):  # 4 consecutive stripes per group
                        pst = ps_t.tile([128, 512], f32, tag="pst")
                        for k in range(4):
                            s = g * 4 + k
                            # matmul-as-transpose (keeps HAM warm)
                            nc.tensor.matmul(
                                pst[:, k * 128 : (k + 1) * 128],
                                xs[s][:, ct * 128 : (ct + 1) * 128],
                                ident[:],
                                start=True,
                                stop=True,
                            )
                        # stage 1: plain evac psum -> tmp (pixel (8g+il, j) at
                        # offset il*64+j)
                        tmp = tpool.tile([128, 512], bf16, tag="tmp")
                        nc.scalar.activation(out=tmp[:], in_=pst[:], func=Copy, scale=1.0)
                        # stage 2: SBUF->SBUF transposing scatter into Q:
                        # x(8g+il, j) -> Q row j+1, col 8g+il+1
                        nc.scalar.activation(
                            out=Qv[:, 1:65, 8 * g + 1 : 8 * g + 9].rearrange(
                                "p j il -> p il j"
                            ),
                            in_=tmp.rearrange("p (il j) -> p il j", il=8),
                            func=Copy,
                            scale=1.0,
                        )
                    # pads: Q row 65 = V[:, 64] = tok0 (V rows 0..64)
                    nc.scalar.activation(
                        out=Qv[:, 65:66, 1:66],
                        in_=zeros[:, 0:65],
                        func=Ident,
                        scale=1.0,
                        bias=tokt[:, 2 * ct : 2 * ct + 1],
                    )
                    # Q col 65 = V[64, :] = tok_{j%2}; Q row p=j+1
                    Qm = Q.rearrange("p (m x) -> p m x", x=2 * RS)
                    nc.scalar.activation(  # odd rows 1..65 (j even) -> tok0
                        out=Qm[:, 0:33, RS + 65 : RS + 66],
                        in_=zeros[:, 0:33],
                        func=Ident,
                        scale=1.0,
                        bias=tokt[:, 2 * ct : 2 * ct + 1],
                    )
                    nc.scalar.activation(  # even rows 2..64 (j odd) -> tok1
                        out=Qm[:, 1:33, 65:66],
                        in_=zeros[:, 0:32],
                        func=Ident,
                        scale=1.0,
                        bias=tokt[:, 2 * ct + 1 : 2 * ct + 2],
                    )

                    # -------- phase B: conv taps (natural z order) ----------
                    z = zpool.tile([128, L], bf16, tag="z")
                    z_tiles.append(z)
                    for n in range(8):
                        pz = ps_z.tile([128, 512], f32, tag="pz")
                        for t, (di, dj) in enumerate(TAPS):
                            # element (il, j): V[j+di, i+dj] = Q[(i+dj)+1, (j+di)+1]
                            rhs = Qv[
                                :,
                                8 * n + dj + 1 : 8 * n + dj + 9,
                                di + 1 : di + 65,
                            ]
                            nc.tensor.matmul(
                                pz[:],
                                diag[t],
                                rhs,
                                start=(t == 0),
                                stop=(t == 8),
                            )
                        # z = conv + bias (natural order; residual in phase C)
                        nc.vector.tensor_scalar_add(
                            out=z[:, n * 512 : (n + 1) * 512],
                            in0=pz[:],
                            scalar1=cb[:, ct : ct + 1],
                        )

                # -------- phase C: transpose back, residual, store ----------
                for s in range(NSTRIPE):
                    p2 = ps_o.tile([128, 1024], f32, tag="p2")
                    for ct in range(NCT):
                        nc.tensor.matmul(
                            p2[:, ct * 128 : (ct + 1) * 128],
                            z_tiles[ct][:, s * 128 : (s + 1) * 128],
                            ident[:],
                            start=True,
                            stop=True,
                        )
                    ob = opool.tile([128, C], f32, tag="ob")
                    nc.vector.tensor_add(out=ob[:], in0=p2[:], in1=xs[s][:])
                    nc.sync.dma_start(
                        out=out_ext[bi, s * 128 : (s + 1) * 128, :], in_=ob[:]
                    )

    nc.finalize()
    return nc


def _get_compiled():
    if "nc" not in _CACHE:
        _CACHE["nc"] = _build_nc()
    return _CACHE["nc"]


def _run(inputs, trace=False):
    from concourse.bass_utils import run_bass_kernel_spmd

    x = np.ascontiguousarray(np.asarray(inputs["x"], dtype=np.float32))
    pad_token = np.ascontiguousarray(np.asarray(inputs["pad_token"], dtype=np.float32))
    conv_w = np.ascontiguousarray(np.asarray(inputs["conv_w"], dtype=np.float32))
    conv_b = np.ascontiguousarray(np.asarray(inputs["conv_b"], dtype=np.float32))
    seq_idx = np.asarray(inputs["seq_idx"]).astype(np.int64)

    nc = _get_compiled()
    in_maps = []
    for k in range(NCORES):
        in_maps.append(
            {
                "x": x[k * BPC : (k + 1) * BPC],
                "pad_token": pad_token,
                "conv_w": conv_w,
                "conv_b": conv_b,
            }
        )
    res = run_bass_kernel_spmd(nc, in_maps, core_ids=list(range(NCORES)), trace=trace)
    out = np.concatenate([r["out"] for r in res.results], axis=0)

    # The device kernel hardcodes the reference's transpose permutation in its
    # access patterns. If the harness ever supplies a different seq_idx,
    # correct on host: out = x + y[:, seq_idx]  with y = (out_dev - x) at the
    # hardcoded permutation undone.
    exp = _expected_seq_idx()
    if not np.array_equal(seq_idx, exp):
        y = (out - x)[:, np.argsort(exp), :]
        out = x + y[:, seq_idx, :]

    return out, getattr(res, "exec_time_ns", None)


def kernel(**inputs) -> np.ndarray:
    out, _ = _run(inputs, trace=False)
    return out



# revision 5
# speedup vs baseline: 1.2639x; 1.2639x over previous
"""Trainium2 Bass kernel for nn_AdapterAttnForMamba (depthwise 3x3 conv over a
pad-token-augmented 66x66 image + spatial-transpose permutation + residual).

Math (per batch b, channel c), derived from the reference:
  out(i,j) = x(i,j) + y(j,i) + bias_c
  y(r,s)   = sum_{a,b in 0..2} w[c,a,b] * V[r+a-1, s+b-1]
  V        = 65x65 "virtual" image: V[r<64, s<64] = x(r,s); V[r<64, 64] = tok0;
             V[64, s] = tok_{s%2}; zero outside (SAME conv padding).

Strategy: pure data parallel over batch (2 batches / core, 8 cores).
Channels-on-partitions layout obtained via TensorE transposes; conv taps are
diagonal matmuls accumulating in PSUM, with tap shifts folded into free-dim AP
offsets into the padded SBUF image P (bf16: moving operand streams 1 col/cyc
at 2.4 GHz vs ~half rate for f32r).  P is kept in NATURAL orientation (no
transposed scatter -- the phase A evac is a contiguous-inner strided copy
straight from PSUM), and the seq_idx spatial transpose is folded into phase
C's LDWEIGHTS access pattern instead.  A second shifted copy Ps (P offset by
one element) is evac'd in the same way so every tap's rhs AP starts at a
4-byte-aligned offset (bf16 odd-element offsets hit an AP bug).

Pipeline per (batch bi, channel-block ct of 128):
  x stripes [128pix,1024ch] --(SWDGE cast f32->bf16)--> SBUF
  TensorE transpose -> PSUM [128ch,128pix] -> ACT copies into P and Ps
  9 diag-matmuls (bf16) accumulate conv into PSUM chunks [128,512]
  DVE evac + conv_b bias -> u[ct] (bf16, natural y(r,s) order, r-major)
  TensorE transpose back with permuted lhsT AP -> PSUM [128pix,1024ch]
  DVE adds residual x stripe; DMA f32 to DRAM out.
"""

import os
import sys

import numpy as np

for _p in ("/opt/trn_rl_repo", "/root/.axon_site/_ro/trn_rl_repo"):
    if os.path.isdir(_p) and _p not in sys.path:
        sys.path.append(_p)

B, H, W, C = 16, 64, 64, 1024
L = H * W  # 4096
NCORES = 8
BPC = B // NCORES  # batches per core
NCT = C // 128  # channel blocks
RS = 80  # P row stride (elements); >= 67 and 16-aligned
PROWS = 66  # P rows: image rows -1..64 (+1 ring)
PSZ = PROWS * RS
NSTRIPE = L // 128  # 32 pixel stripes per batch
TAPS = [(di, dj) for di in (-1, 0, 1) for dj in (-1, 0, 1)]

_CACHE = {}


def _expected_seq_idx():
    return np.arange(L).reshape(H, W).T.reshape(-1)


def _build_nc():
    import concourse.mybir as mybir
    from concourse import bacc
    from concourse.masks import make_identity
    from concourse.tile import TileContext

    f32 = mybir.dt.float32
    bf16 = mybir.dt.bfloat16
    Copy = mybir.ActivationFunctionType.Copy
    Ident = mybir.ActivationFunctionType.Identity

    nc = bacc.Bacc(None, target_bir_lowering=False)
    x_ext = nc.declare_dram_parameter("x", [BPC, L, C], f32, isOutput=False)
    tok_ext = nc.declare_dram_parameter("pad_token", [1, C, 2], f32, isOutput=False)
    w_ext = nc.declare_dram_parameter("conv_w", [C, 1, 3, 3], f32, isOutput=False)
    b_ext = nc.declare_dram_parameter("conv_b", [C], f32, isOutput=False)
    out_ext = nc.declare_dram_parameter("out", [BPC, L, C], f32, isOutput=True)

    with TileContext(nc) as tc:
        with (
            tc.tile_pool(name="const", bufs=1) as constp,
            tc.tile_pool(name="xpool", bufs=32) as xpool,
            tc.tile_pool(name="zpool", bufs=8) as zpool,
            tc.tile_pool(name="qpool", bufs=2) as qpool,
            tc.tile_pool(name="opool", bufs=2) as opool,
            tc.tile_pool(name="dpool", bufs=12) as dpool,
            tc.tile_pool(name="ps_t", bufs=2, space="PSUM") as ps_t,
            tc.tile_pool(name="ps_z", bufs=2, space="PSUM") as ps_z,
            tc.tile_pool(name="ps_o", bufs=2, space="PSUM") as ps_o,
        ):
            # ---- constants ----
            ident = constp.tile([128, 128], bf16, tag="ident")
            make_identity(nc, ident)
            zeros = constp.tile([128, 128], bf16, tag="zeros")
            nc.vector.memset(zeros[:], 0.0)

            wt = constp.tile([128, 9 * NCT], f32, tag="wt")
            cb = constp.tile([128, NCT], f32, tag="cb")
            tokt = constp.tile([128, 2 * NCT], f32, tag="tokt")
            nc.sync.dma_start(
                out=wt.rearrange("p (ct t) -> p ct t", t=9),
                in_=w_ext.rearrange("(ct p) a k l -> p ct (a k l)", ct=NCT),
            )
            nc.sync.dma_start(
                out=cb[:],
                in_=b_ext.rearrange("(ct p) -> p ct", ct=NCT),
            )
            nc.sync.dma_start(
                out=tokt.rearrange("p (ct two) -> p ct two", two=2),
                in_=tok_ext.rearrange("a (ct p) two -> p ct (a two)", ct=NCT),
            )

            # ---- main loops ----
            # P[c, (r+1)*RS + (s+1)] = V[r, s] (natural orientation);
            # Ps[c, k] = P[c, k+1] (the 4B-alignment shadow copy).
            for bi in range(BPC):
                xs = []
                for s in range(NSTRIPE):
                    xt = xpool.tile([128, C], bf16, tag="x")
                    # SWDGE dma casts f32 -> bf16; plain natural stripe
                    nc.gpsimd.dma_start(
                        out=xt[:], in_=x_ext[bi, s * 128 : (s + 1) * 128, :]
                    )
                    xs.append(xt)

                z_tiles = []
                for ct in range(NCT):
                    # -------- phase A: build padded image P (+ shadow Ps) ---
                    P = qpool.tile([128, PSZ], bf16, tag="P")
                    Ps = qpool.tile([128, PSZ], bf16, tag="Ps")
                    Pv = P.rearrange("p (r c) -> p r c", c=RS)
                    Psv = Ps.rearrange("p (r c) -> p r c", c=RS)
                    # ring zeros: V[-1, *] and V[*, -1]
                    nc.scalar.activation(
                        out=Pv[:, 0:1, 0:67], in_=zeros[:, 0:67], func=Copy, scale=1.0
                    )
                    nc.scalar.activation(
                        out=Pv[:, 1:66, 0:1], in_=zeros[:, 0:65], func=Copy, scale=1.0
                    )
                    nc.scalar.activation(
                        out=Psv[:, 0:1, 0:66], in_=zeros[:, 0:66], func=Copy, scale=1.0
                    )
                    diag = []
                    for t in range(9):
                        d = dpool.tile([128, 128], bf16, tag="diag")
                        nc.vector.tensor_scalar_mul(
                            out=d[:],
                            in0=ident[:],
                            scalar1=wt[:, ct * 9 + t : ct * 9 + t + 1],
                        )
                        diag.append(d)
                    for g in range(8):  # 4 consecutive stripes per group
                        pst = ps_t.tile([128, 512], f32, tag="pst")
                        for k in range(4):
                            s = g * 4 + k
                            nc.tensor.matmul(
                                pst[:, k * 128 : (k + 1) * 128],
                                xs[s][:, ct * 128 : (ct + 1) * 128],
                                ident[:],
                                start=True,
                                stop=True,
                            )
                        # pst[c, il*64+j] = x(8g+il, j) -> P rows 8g+1..8g+9,
                        # cols 1..65 (contiguous inner runs of 64)
                        nc.scalar.activation(
                            out=Pv[:, 8 * g + 1 : 8 * g + 9, 1:65],
                            in_=pst.rearrange("p (il j) -> p il j", il=8),
                            func=Copy,
                            scale=1.0,
                        )
                        nc.scalar.activation(
                            out=Psv[:, 8 * g + 1 : 8 * g + 9, 0:64],
                            in_=pst.rearrange("p (il j) -> p il j", il=8),
                            func=Copy,
                            scale=1.0,
                        )
                    # pads: V[r<64, 64] = tok0  (strided col writes)
                    nc.scalar.activation(
                        out=Pv[:, 1:65, 65:66],
                        in_=zeros[:, 0:64],
                        func=Ident,
                        scale=1.0,
                        bias=tokt[:, 2 * ct : 2 * ct + 1],
                    )
                    nc.scalar.activation(
                        out=Psv[:, 1:65, 64:65],
                        in_=zeros[:, 0:64],
                        func=Ident,
                        scale=1.0,
                        bias=tokt[:, 2 * ct : 2 * ct + 1],
                    )
                    # pads: V[64, s] = tok_{s%2} for s=0..64 (row 65)
                    Pm = P.rearrange("p (m x) -> p m x", x=2)
                    Pms = Ps.rearrange("p (m x) -> p m x", x=2)
                    r65 = 65 * RS  # even
                    nc.scalar.activation(  # P: even s -> cols r65+1,+3,.. (33)
                        out=Pm[:, r65 // 2 : r65 // 2 + 33, 1:2],
                        in_=zeros[:, 0:33],
                        func=Ident,
                        scale=1.0,
                        bias=tokt[:, 2 * ct : 2 * ct + 1],
                    )
                    nc.scalar.activation(  # P: odd s -> cols r65+2,+4,.. (32)
                        out=Pm[:, r65 // 2 + 1 : r65 // 2 + 33, 0:1],
                        in_=zeros[:, 0:32],
                        func=Ident,
                        scale=1.0,
                        bias=tokt[:, 2 * ct + 1 : 2 * ct + 2],
                    )
                    nc.scalar.activation(  # Ps: even s -> cols r65,+2,.. (33)
                        out=Pms[:, r65 // 2 : r65 // 2 + 33, 0:1],
                        in_=zeros[:, 0:33],
                        func=Ident,
                        scale=1.0,
                        bias=tokt[:, 2 * ct : 2 * ct + 1],
                    )
                    nc.scalar.activation(  # Ps: odd s -> cols r65+1,+3,.. (32)
                        out=Pms[:, r65 // 2 : r65 // 2 + 32, 1:2],
                        in_=zeros[:, 0:32],
                        func=Ident,
                        scale=1.0,
                        bias=tokt[:, 2 * ct + 1 : 2 * ct + 2],
                    )

                    # -------- phase B: conv taps -> z (permuted) order -------
                    # chunk n computes y(r,s) for rows r=8n..8n+8 (s-inner);
                    # tap (di,dj) reads P[8n+di+1 .. +9, dj+1 : dj+65];
                    # dj==0 starts at an odd element offset -> read Ps instead.
                    # The evac scatters into z[c, s*64+r] (seq_idx transpose),
                    # so phase C's LDWEIGHTS can use contiguous slices.
                    z = zpool.tile([128, L], bf16, tag="z")
                    z_tiles.append(z)
                    zv = z.rearrange("p (s r) -> p s r", r=64)
                    for n in range(8):
                        pz = ps_z.tile([128, 512], f32, tag="pz")
                        for t, (di, dj) in enumerate(TAPS):
                            if dj == 0:
                                rhs = Psv[
                                    :, 8 * n + di + 1 : 8 * n + di + 9, 0:64
                                ]
                            else:
                                rhs = Pv[
                                    :, 8 * n + di + 1 : 8 * n + di + 9,
                                    dj + 1 : dj + 65,
                                ]
                            nc.tensor.matmul(
                                pz[:],
                                diag[t],
                                rhs,
                                start=(t == 0),
                                stop=(t == 8),
                            )
                        # z[c, s*64 + (8n+dr)] = pz[c, dr*64+s] + bias
                        nc.vector.tensor_scalar_add(
                            out=zv[:, :, 8 * n : 8 * n + 8].rearrange(
                                "p s r -> p r s"
                            ),
                            in0=pz[:],
                            scalar1=cb[:, ct : ct + 1],
                        )

                # -------- phase C: transpose back, residual, store ----------
                # z is already in permuted order, so lhsT is contiguous.
                for s in range(NSTRIPE):
                    p2 = ps_o.tile([128, 1024], f32, tag="p2")
                    for ct in range(NCT):
                        nc.tensor.matmul(
                            p2[:, ct * 128 : (ct + 1) * 128],
                            z_tiles[ct][:, s * 128 : (s + 1) * 128],
                            ident[:],
                            start=True,
                            stop=True,
                        )
                    ob = opool.tile([128, C], f32, tag="ob")
                    nc.vector.tensor_add(out=ob[:], in0=p2[:], in1=xs[s][:])
                    nc.sync.dma_start(
                        out=out_ext[bi, s * 128 : (s + 1) * 128, :], in_=ob[:]
                    )

    nc.finalize()
    return nc


def _get_compiled():
    if "nc" not in _CACHE:
        _CACHE["nc"] = _build_nc()
    return _CACHE["nc"]


def _run(inputs, trace=False):
    from concourse.bass_utils import run_bass_kernel_spmd

    x = np.ascontiguousarray(np.asarray(inputs["x"], dtype=np.float32))
    pad_token = np.ascontiguousarray(np.asarray(inputs["pad_token"], dtype=np.float32))
    conv_w = np.ascontiguousarray(np.asarray(inputs["conv_w"], dtype=np.float32))
    conv_b = np.ascontiguousarray(np.asarray(inputs["conv_b"], dtype=np.float32))
    seq_idx = np.asarray(inputs["seq_idx"]).astype(np.int64)

    nc = _get_compiled()
    in_maps = []
    for k in range(NCORES):
        in_maps.append(
            {
                "x": x[k * BPC : (k + 1) * BPC],
                "pad_token": pad_token,
                "conv_w": conv_w,
                "conv_b": conv_b,
            }
        )
    res = run_bass_kernel_spmd(nc, in_maps, core_ids=list(range(NCORES)), trace=trace)
    out = np.concatenate([r["out"] for r in res.results], axis=0)

    # The device kernel hardcodes the reference's transpose permutation in its
    # access patterns. If the harness ever supplies a different seq_idx,
    # correct on host: out = x + y[:, seq_idx]  with y = (out_dev - x) at the
    # hardcoded permutation undone.
    exp = _expected_seq_idx()
    if not np.array_equal(seq_idx, exp):
        y = (out - x)[:, np.argsort(exp), :]
        out = x + y[:, seq_idx, :]

    return out, getattr(res, "exec_time_ns", None)


def kernel(**inputs) -> np.ndarray:
    out, _ = _run(inputs, trace=False)
    return out
